# revision 1
# baseline (speedup 1.0000x reference)
"""Causal multi-head attention (B=4, T=2048, C=1024, H=16, D=64) on 8 trn2 cores.

Sharding: data-parallel over batch (4) x tensor-parallel over heads (2 groups
of 8). Core c handles batch c//2, head group c%2. Each core computes

    qkvT/V for its 8 heads -> causal attention -> partial y = attn @ w_proj_rows

and the host sums the two per-batch partial outputs (the tensor-parallel
reduce).

Device kernel (per core), all matmuls fp32r (full PE rate at N=512):
  A: transpose x [T,C] -> xT [C,T] via PE transpose (128x128 tiles)
  B: V = x @ w_v      -> vaug [T, 8 heads x (64+1)] bf16, ones col for sums
  C: qkT = (x @ w_qk)^T stored [feat, T] (q feats 0..511, k feats 512..1023)
  D: per (Tq-chunk 512, head): ST[tk,tq] = KT_blk.T @ QT  (K=64 matmul)
     PT = exp(scale*ST) (ACT, psum->sbuf bf16); causal mask on diagonal
     blocks (precomputed bf16 masks, DVE mult); outT[65,512] += Vaug.T @ PT
     (ones col makes row 64 the softmax denominators); normalize rows 0..63
     by 1/row64 and write into attn [C_h, T] (the proj lhsT layout).
  E: y[T,1024] = attn.T @ w_proj_shard, accumulate over C_h=512.
"""

import numpy as np

B, T, C, H, D = 4, 2048, 1024, 16, 64
HPG = 8            # heads per group (per core)
CG = HPG * D       # 512 features per group
SCALE = float(D) ** -0.5
NT = T // 128      # 16 T tiles
NKC = C // 128     # 8 contraction tiles over C
NQ = T // 512      # 4 Tq chunks
NMT = 8            # qkT feature tiles (1024 feats)

_PROG = None       # cached compiled Bass program


def _build_program(loop_n=1, phases="ABCDE", dmode="full"):
    import contextlib
    import concourse.bacc as bacc
    import concourse.mybir as mybir
    import concourse.tile as tile
    from concourse.masks import make_identity

    F32 = mybir.dt.float32
    F32R = mybir.dt.float32r
    BF16 = mybir.dt.bfloat16
    Exp = mybir.ActivationFunctionType.Exp

    nc = bacc.Bacc("TRN2", target_bir_lowering=False, debug=False)

    with tile.TileContext(nc) as tc:
        loop_cm = tc.For_i(0, loop_n, 1) if loop_n > 1 \
            else contextlib.nullcontext()
        with loop_cm, \
             tc.tile_pool(name="dram", bufs=1, space="DRAM") as dram, \
             tc.tile_pool(name="persist", bufs=1) as persist:
            x_d = dram.tile([T, C], F32R, kind="ExternalInput", name="x",
                            uniquify=False)
            wqk_d = dram.tile([C, 2 * CG], F32R, kind="ExternalInput",
                              name="wqk", uniquify=False)
            wv_d = dram.tile([C, CG], F32R, kind="ExternalInput", name="wv",
                             uniquify=False)
            wproj_d = dram.tile([CG, C], F32R, kind="ExternalInput",
                                name="wproj", uniquify=False)
            y_d = dram.tile([T, C], F32, kind="ExternalOutput", name="y",
                            uniquify=False)

            # persistent SBUF
            qkT = persist.tile([128, NMT, T], F32R)       # 64 KB/part
            vaug = persist.tile([128, NT, HPG, D + 1], BF16)  # 16.25 KB/part
            ident = persist.tile([128, 128], F32)
            tri = persist.tile([128, 128], BF16)          # causal triangle

            make_identity(nc, ident[:])
            # tri: keep[i, j] iff j - i >= 0  (the 128x128 diagonal triangle)
            nc.vector.memset(tri[:], 1.0)
            nc.gpsimd.affine_select(
                out=tri[:], in_=tri[:],
                compare_op=mybir.AluOpType.is_ge,
                fill=0.0, base=0, channel_multiplier=-1,
                pattern=[[1, 128]],
            )
            # ones column of vaug
            nc.vector.memset(vaug[:, :, :, D:D + 1], 1.0)

            # ---------------- phases A-C ----------------
            with tc.tile_pool(name="abc", bufs=1) as abc, \
                 tc.tile_pool(name="x_pool", bufs=3) as x_pool, \
                 tc.tile_pool(name="wqk_pool", bufs=10) as wqk_pool:
                xT = abc.tile([128, NKC, T], F32R)        # 64 KB/part
                wv_sb = abc.tile([128, NKC, CG], F32R)    # 16 KB/part
                nc.sync.dma_start(
                    out=wv_sb[:],
                    in_=wv_d[:].rearrange("(k p) n -> p k n", p=128))

                # A: transpose x into xT
                with tc.tile_pool(name="tp_ps", bufs=4, space="PSUM") as tp_ps:
                    for tt in range(NT if "A" in phases else 0):
                        x_row = x_pool.tile([128, C], F32R, tag="xrow")
                        nc.sync.dma_start(
                            out=x_row[:], in_=x_d[tt * 128:(tt + 1) * 128, :])
                        for kc in range(NKC):
                            tp = tp_ps.tile([128, 128], F32, tag="tp")
                            nc.tensor.transpose(
                                tp[:],
                                x_row[:, kc * 128:(kc + 1) * 128].bitcast(F32),
                                ident[:])
                            nc.vector.tensor_copy(
                                xT[:, kc, tt * 128:(tt + 1) * 128], tp[:])

                with tc.tile_pool(name="mm_ps", bufs=4, space="PSUM") as mm_ps:
                    # B: V = x @ wv -> vaug (bf16) with per-head 65-col slots
                    for tt in range(NT if "B" in phases else 0):
                        psv = mm_ps.tile([128, CG], F32, tag="mm")
                        for kc in range(NKC):
                            nc.tensor.matmul(
                                psv[:],
                                xT[:, kc, tt * 128:(tt + 1) * 128],
                                wv_sb[:, kc, :],
                                start=(kc == 0), stop=(kc == NKC - 1))
                        nc.vector.tensor_copy(
                            vaug[:, tt, :, 0:D],
                            psv[:].rearrange("p (h d) -> p h d", h=HPG))

                    # C: qkT = (x @ wqk)^T  -> [feat, T]
                    for m in range(NMT if "C" in phases else 0):
                        wtiles = []
                        for kc in range(NKC):
                            wt = wqk_pool.tile([128, 128], F32R, tag="wqk")
                            nc.sync.dma_start(
                                out=wt[:],
                                in_=wqk_d[kc * 128:(kc + 1) * 128,
                                          m * 128:(m + 1) * 128])
                            wtiles.append(wt)
                        for n in range(NQ):
                            psq = mm_ps.tile([128, 512], F32, tag="mm")
                            for kc in range(NKC):
                                nc.tensor.matmul(
                                    psq[:],
                                    wtiles[kc][:],
                                    xT[:, kc, n * 512:(n + 1) * 512],
                                    start=(kc == 0), stop=(kc == NKC - 1))
                            nc.vector.tensor_copy(
                                qkT[:, m, n * 512:(n + 1) * 512], psq[:])

            # ---------------- phases D-E ----------------
            with tc.tile_pool(name="de", bufs=1) as de, \
                 tc.tile_pool(name="pt_pool", bufs=8) as pt_pool, \
                 tc.tile_pool(name="small", bufs=4) as small, \
                 tc.tile_pool(name="y_stage", bufs=3) as y_stage:
                attn = de.tile([128, 4, T], F32R)          # 32 KB/part
                wproj_sb = de.tile([128, 4, C], F32R)      # 16 KB/part
                if dmode == "stpv":
                    dummy_pt = de.tile([128, 512], BF16)
                    nc.vector.memset(dummy_pt[:], 0.001)
                nc.sync.dma_start(
                    out=wproj_sb[:],
                    in_=wproj_d[:].rearrange("(k p) n -> p k n", p=128))

                # D: attention per (Tq chunk, head pair). Even/odd heads sit
                # at partition bases 0/64 of qkT, so their ST matmuls land on
                # disjoint PE row groups and run concurrently when issued
                # back to back. Diagonal blocks (r >= 0) only compute the
                # valid column range [128 r, 512) and mask just the leading
                # 128x128 triangle.
                with tc.tile_pool(name="st_ps", bufs=4, space="PSUM") as st_ps, \
                     tc.tile_pool(name="out_ps", bufs=4, space="PSUM") as out_ps:
                    for qc in range(NQ if "D" in phases else 0):
                        nkb = 4 * qc + 4
                        for half in range(2):
                            heads = [4 * half + j for j in range(4)]
                            outs = {}
                            for h in heads:
                                outs[h] = out_ps.tile(
                                    [D + 1, 512], F32, tag="outp",
                                    name=f"outp_{qc}_{h}")
                            for kb in range(nkb):
                                r = kb - 4 * qc
                                jlo = 128 * r if r > 0 else 0
                                w = 512 - jlo
                                pss = {}
                                for h in heads:
                                    pb = (h % 2) * 64
                                    mq = h // 2
                                    mk = 4 + h // 2
                                    ps_s = st_ps.tile(
                                        [128, 512], F32, tag="st",
                                        name=f"st_{qc}_{h}_{kb}")
                                    nc.tensor.matmul(
                                        ps_s[:, 0:w],
                                        qkT[pb:pb + 64, mk,
                                            kb * 128:(kb + 1) * 128],
                                        qkT[pb:pb + 64, mq,
                                            qc * 512 + jlo:(qc + 1) * 512],
                                        start=True, stop=True)
                                    pss[h] = ps_s
                                pts = {}
                                for h in (heads if dmode == "full" else ()):
                                    pt = pt_pool.tile(
                                        [128, 512], BF16, tag="pt",
                                        name=f"pt_{qc}_{h}_{kb}")
                                    nc.scalar.activation(
                                        pt[:, 0:w], pss[h][:, 0:w], Exp,
                                        scale=SCALE)
                                    if r >= 0:  # leading triangle
                                        nc.vector.tensor_mul(
                                            pt[:, 0:128], pt[:, 0:128],
                                            tri[:])
                                    pts[h] = pt
                                if dmode != "st":
                                    for h in heads:
                                        rhs_pt = (dummy_pt
                                                  if dmode == "stpv"
                                                  else pts[h])
                                        nc.tensor.matmul(
                                            outs[h][:, jlo:512],
                                            vaug[:, kb, h, :],
                                            rhs_pt[:, 0:w],
                                            start=(kb == 0),
                                            stop=(kb == nkb - 1))
                            for h in (heads if dmode == "full" else ()):
                                pb = (h % 2) * 64
                                outp = outs[h]
                                rec = small.tile([1, 512], F32, tag="rec")
                                nc.vector.reciprocal(rec[:],
                                                     outp[D:D + 1, :])
                                bc = small.tile([D, 512], F32, tag="bc")
                                nc.gpsimd.partition_broadcast(bc[:], rec[:])
                                nc.vector.tensor_mul(
                                    attn[pb:pb + 64, h // 2,
                                         qc * 512:(qc + 1) * 512],
                                    outp[0:D, :], bc[:])

                # E: y = attn.T @ wproj
                with tc.tile_pool(name="y_ps", bufs=4, space="PSUM") as y_ps:
                    for tt in range(NT if "E" in phases else 0):
                        for nn in range(2):
                            psy = y_ps.tile([128, 512], F32, tag="y")
                            for kt in range(4):
                                nc.tensor.matmul(
                                    psy[:],
                                    attn[:, kt, tt * 128:(tt + 1) * 128],
                                    wproj_sb[:, kt, nn * 512:(nn + 1) * 512],
                                    start=(kt == 0), stop=(kt == 3))
                            ys = y_stage.tile([128, 512], F32, tag="ys")
                            nc.vector.tensor_copy(ys[:], psy[:])
                            nc.sync.dma_start(
                                out=y_d[tt * 128:(tt + 1) * 128,
                                        nn * 512:(nn + 1) * 512],
                                in_=ys[:])

    nc.compile()
    return nc


def _get_program():
    global _PROG
    if _PROG is None:
        _PROG = _build_program()
    return _PROG


def kernel(x, w_qkv, w_proj):
    from concourse.bass_utils import run_bass_kernel_spmd

    x = np.asarray(x, dtype=np.float32)
    w_qkv = np.asarray(w_qkv, dtype=np.float32)
    w_proj = np.asarray(w_proj, dtype=np.float32)

    in_maps = []
    for c in range(8):
        b, g = c // 2, c % 2
        wq = w_qkv[:, g * CG:(g + 1) * CG]
        wk = w_qkv[:, C + g * CG:C + (g + 1) * CG]
        wv = w_qkv[:, 2 * C + g * CG:2 * C + (g + 1) * CG]
        in_maps.append({
            "x": np.ascontiguousarray(x[b]),
            "wqk": np.ascontiguousarray(np.concatenate([wq, wk], axis=1)),
            "wv": np.ascontiguousarray(wv),
            "wproj": np.ascontiguousarray(w_proj[g * CG:(g + 1) * CG, :]),
        })

    nc = _get_program()
    res = run_bass_kernel_spmd(nc, in_maps, core_ids=list(range(8)))

    out = np.empty((B, T, C), dtype=np.float32)
    for b in range(B):
        out[b] = res.results[2 * b]["y"] + res.results[2 * b + 1]["y"]
    return out

